# revision 10
# baseline (speedup 1.0000x reference)
"""PointNet MLP (3 x conv1x1+BN+ReLU, final valid-mask) on 8 TRN2 cores.

Sharding: compacted-column parallel. The valid mask keeps ~70% of the
4096*128 = 524288 point-neighbor columns; masked columns are exactly 0 in
the reference output. Host gathers the valid columns, splits them evenly
across 8 cores, device computes only those, host scatters into zeros.

Numerics: plain fp16 weights/activations with f32 PSUM accumulation and
fp16 output (end-to-end rel err ~7e-4 vs the 2e-2 gate).

Device schedule: supersteps of 2048 stream columns (2 block-pair
iterations of 1024), software-pipelined 3 stages deep.  Layers 1/2 use
one joint PSUM tile per superstep (ps1/ps2 [128,1024], single-buffered:
their consumers act1/act2 run FIRST on their engines, so the
write-after-read from the next superstep resolves early); layer 3 uses
per-iteration [128,1024] tiles, double-buffered (its consumer act3 runs
last, so its WAR gets a full superstep of slack).  8 PSUM banks total.
Priority offsets pin the scheduler to:
  PE : mm1 pair(s) | mm2 pair(s-1) | mm3 quad(s-2)  - streams 8
       back-to-back matmuls per superstep at the full column rate
       (a matmul that stalls on a semaphore pays a ~300ns restart)
  ACT: act1j(s) Relu(ps1)->fp16 | act3(even iter of s-2) + bias b3
  DVE: act2j(s-1) max(ps2+b2cat,0)->fp16 | act3(odd iter of s-2)
b1 is folded into mm1 via a ones row (K=7); BN folded on host (f64).
One fp16 [128,1024] DMA out per iteration; host casts and scatters.
"""

import numpy as np

try:
    import concourse.bass as bass
except ImportError:
    import sys

    sys.path.insert(0, "/opt/trn_rl_repo")
    import concourse.bass as bass

import concourse.bacc as bacc

import concourse.mybir as mybir
from concourse import tile
from concourse.bass_utils import run_bass_kernel_spmd

F32 = mybir.dt.float32
F16 = mybir.dt.float16

N_CORES = 8
NPOINT, KNN = 4096, 128
NCOLS = NPOINT * KNN
M = 512
ITER_COLS = 2 * M        # 1024 stream columns per iteration
SS_COLS = 2 * ITER_COLS  # 2048 stream columns per superstep
EPS = 1e-5

_NC_CACHE = {}


def _build_nc(iters):
    assert iters % 2 == 0
    n_ss = iters // 2
    nc = bacc.Bacc("TRN2", target_bir_lowering=False)
    xp_d = nc.declare_dram_parameter("xp", [14, iters * M], F16, isOutput=False)
    w1_d = nc.declare_dram_parameter("lhsT1", [14, 128], F16, isOutput=False)
    w2_d = nc.declare_dram_parameter("lhsT2", [128, 128], F16, isOutput=False)
    w3_d = nc.declare_dram_parameter("lhsT3", [128, 128], F16, isOutput=False)
    bias_d = nc.declare_dram_parameter("biases", [128, 2], F32, isOutput=False)
    out_d = nc.declare_dram_parameter("out", [128, iters * ITER_COLS], F16,
                                      isOutput=True)

    add = mybir.AluOpType.add
    vmax = mybir.AluOpType.max
    relu_fn = mybir.ActivationFunctionType.Relu

    with tile.TileContext(nc) as tc:
        with (
            tc.tile_pool(name="const", bufs=1) as cpool,
            tc.tile_pool(name="xpool", bufs=1) as xpool,
            tc.tile_pool(name="ypool", bufs=3) as ypool,
            tc.tile_pool(name="opool", bufs=4) as opool,
            tc.tile_pool(name="pspool", bufs=1, space="PSUM") as pspool,
            tc.tile_pool(name="ps3pool", bufs=2, space="PSUM") as ps3pool,
        ):
            w1_sb = cpool.tile([39, 128], F16, tag="w1", name="w1_sb")
            w2_sb = cpool.tile([128, 128], F16, tag="w2", name="w2_sb")
            w3_sb = cpool.tile([128, 128], F16, tag="w3", name="w3_sb")
            bias_sb = cpool.tile([128, 2], F32, tag="bias", name="bias_sb")
            nc.sync.dma_start(w1_sb[0:7, :], w1_d[0:7, :])
            nc.sync.dma_start(w1_sb[32:39, :], w1_d[7:14, :])
            nc.sync.dma_start(w2_sb[:, :], w2_d[:, :])
            nc.sync.dma_start(w3_sb[:, :], w3_d[:, :])
            nc.sync.dma_start(bias_sb[:, :], bias_d[:, :])
            b2_ap = bias_sb[:, 0:1]
            b3_ap = bias_sb[:, 1:2]

            xcols = iters * M
            x_sb = xpool.tile([39, xcols], F16, tag="x", name="x_sb")
            nch = 4
            chunk = -(-xcols // nch)
            for c in range(nch):
                lo = c * chunk
                hi = min(xcols, lo + chunk)
                if hi > lo:
                    nc.sync.dma_start(x_sb[0:7, lo:hi], xp_d[0:7, lo:hi])
                    nc.sync.dma_start(x_sb[32:39, lo:hi], xp_d[7:14, lo:hi])

            T = {}

            def mk(s):
                T[s] = dict(
                    ps1=pspool.tile([128, 2 * M], F32, tag="ps1",
                                    name=f"ps1_{s}"),
                    ps2=pspool.tile([128, 2 * M], F32, tag="ps2",
                                    name=f"ps2_{s}"),
                    ps3a=ps3pool.tile([128, 2 * M], F32, tag="ps3",
                                      name=f"ps3a_{s}"),
                    ps3b=ps3pool.tile([128, 2 * M], F32, tag="ps3",
                                      name=f"ps3b_{s}"),
                    hi1=ypool.tile([128, 2 * M], F16, tag="hi1",
                                   name=f"hi1_{s}"),
                    hi2=ypool.tile([128, 2 * M], F16, tag="hi2",
                                   name=f"hi2_{s}"),
                    oba=opool.tile([128, 2 * M], F16, tag="ob",
                                   name=f"oba_{s}"),
                    obb=opool.tile([128, 2 * M], F16, tag="ob",
                                   name=f"obb_{s}"),
                )

            for s in range(n_ss + 2):
                if s < n_ss:
                    mk(s)
                    d = T[s]
                    c0 = s * 2 * M
                    with tc.high_priority(offset=60):
                        nc.tensor.matmul(d["ps1"][:, 0:M], w1_sb[0:7, :],
                                         x_sb[0:7, c0 : c0 + M])
                        nc.tensor.matmul(d["ps1"][:, M : 2 * M], w1_sb[32:39, :],
                                         x_sb[32:39, c0 + M : c0 + 2 * M])
                if 1 <= s <= n_ss:
                    d = T[s - 1]
                    with tc.high_priority(offset=40):
                        nc.tensor.matmul(d["ps2"][:, 0:M], w2_sb[:, :],
                                         d["hi1"][:, 0:M])
                        nc.tensor.matmul(d["ps2"][:, M : 2 * M], w2_sb[:, :],
                                         d["hi1"][:, M : 2 * M])
                if s >= 2:
                    d = T[s - 2]
                    nc.tensor.matmul(d["ps3a"][:, 0:M], w3_sb[0:64, :],
                                     d["hi2"][0:64, 0:M])
                    nc.tensor.matmul(d["ps3a"][:, M : 2 * M], w3_sb[64:128, :],
                                     d["hi2"][64:128, 0:M])
                    nc.tensor.matmul(d["ps3b"][:, 0:M], w3_sb[0:64, :],
                                     d["hi2"][0:64, M : 2 * M])
                    nc.tensor.matmul(d["ps3b"][:, M : 2 * M], w3_sb[64:128, :],
                                     d["hi2"][64:128, M : 2 * M])

                if s < n_ss:
                    d = T[s]
                    with tc.high_priority(offset=60):
                        nc.scalar.activation(d["hi1"][:, :], d["ps1"][:, :],
                                             relu_fn)
                if 1 <= s <= n_ss:
                    d = T[s - 1]
                    with tc.high_priority(offset=40):
                        nc.vector.tensor_scalar(d["hi2"][:, :], d["ps2"][:, :],
                                                b2_ap, 0.0, add, vmax)
                if s >= 2:
                    d = T[s - 2]
                    nc.scalar.activation(d["oba"][:, :], d["ps3a"][:, :],
                                         relu_fn, bias=b3_ap)
                    nc.vector.tensor_scalar(d["obb"][:, :], d["ps3b"][:, :],
                                            b3_ap, 0.0, add, vmax)
                    o0 = (s - 2) * SS_COLS
                    nc.sync.dma_start(out_d[:, o0 : o0 + 2 * M],
                                      d["oba"][:, :])
                    nc.sync.dma_start(out_d[:, o0 + 2 * M : o0 + 4 * M],
                                      d["obb"][:, :])
                    del T[s - 2]

    nc.compile()
    return nc


def _get_nc(iters):
    if iters not in _NC_CACHE:
        _NC_CACHE[iters] = _build_nc(iters)
    return _NC_CACHE[iters]


def _fold_bn(W, b, gamma, beta, mean, var):
    inv = gamma.astype(np.float64) / np.sqrt(var.astype(np.float64) + EPS)
    Wp = (W.astype(np.float64) * inv[:, None]).astype(np.float32)
    bp = ((b.astype(np.float64) - mean.astype(np.float64)) * inv
          + beta.astype(np.float64)).astype(np.float32)
    return Wp, bp


def _prepare(inputs):
    gp = np.asarray(inputs["grouped_pc"], dtype=np.float32)
    valid = np.asarray(inputs["valid"], dtype=np.float32)

    Wp1, bp1 = _fold_bn(*(np.asarray(inputs[k], dtype=np.float32)
                          for k in ("W1", "b1", "gamma1", "beta1", "mean1", "var1")))
    Wp2, bp2 = _fold_bn(*(np.asarray(inputs[k], dtype=np.float32)
                          for k in ("W2", "b2", "gamma2", "beta2", "mean2", "var2")))
    Wp3, bp3 = _fold_bn(*(np.asarray(inputs[k], dtype=np.float32)
                          for k in ("W3", "b3", "gamma3", "beta3", "mean3", "var3")))

    lhsT1 = np.zeros((14, 128), np.float16)
    lhsT1[0:3, 0:64] = Wp1.T
    lhsT1[3:6, 64:128] = Wp1.T
    lhsT1[6, 0:64] = bp1
    lhsT1[6, 64:128] = bp1
    lhsT1[7:14] = lhsT1[0:7]

    lhsT2 = np.zeros((128, 128), np.float16)
    lhsT2[0:64, 0:64] = Wp2.T
    lhsT2[64:128, 64:128] = Wp2.T

    lhsT3 = np.zeros((128, 128), np.float16)
    lhsT3[0:64, :] = Wp3.T
    lhsT3[64:128, :] = Wp3.T

    biases = np.zeros((128, 2), np.float32)
    biases[:, 0] = np.concatenate([bp2, bp2])
    biases[:, 1] = bp3

    x = gp[0].reshape(3, NCOLS)
    vidx = np.flatnonzero(valid.reshape(NCOLS) > 0.5)
    V = len(vidx)
    Vc = -(-V // N_CORES)
    iters = max(2, 2 * (-(-Vc // SS_COLS)))
    cap = iters * ITER_COLS

    xv = x[:, vidx].astype(np.float16)

    in_maps = []
    for c in range(N_CORES):
        lo_i = c * Vc
        hi_i = min((c + 1) * Vc, V)
        n = max(0, hi_i - lo_i)
        xa = np.zeros((3, cap), np.float16)
        if n:
            xa[:, :n] = xv[:, lo_i:hi_i]
        xr = xa.reshape(3, iters, 2, M)
        xp = np.ones((14, iters, M), np.float16)
        xp[0:3] = xr[:, :, 0, :]
        xp[3:6] = xr[:, :, 1, :]
        xp[7:14] = xp[0:7]
        in_maps.append(
            {
                "xp": np.ascontiguousarray(xp.reshape(14, iters * M)),
                "lhsT1": lhsT1,
                "lhsT2": lhsT2,
                "lhsT3": lhsT3,
                "biases": biases,
            }
        )
    return in_maps, vidx, V, Vc, iters


def _gather(results, vidx, V, Vc):
    stream = np.empty((128, V), np.float32)
    for c in range(N_CORES):
        lo_i = c * Vc
        hi_i = min((c + 1) * Vc, V)
        if hi_i <= lo_i:
            break
        stream[:, lo_i:hi_i] = results[c]["out"][:, : hi_i - lo_i]
    full = np.zeros((128, NCOLS), np.float32)
    full[:, vidx] = stream
    return full.reshape(128, NPOINT, KNN)[None]


def run_traced(trace=False, **inputs):
    in_maps, vidx, V, Vc, iters = _prepare(inputs)
    nc = _get_nc(iters)
    res = run_bass_kernel_spmd(nc, in_maps, list(range(N_CORES)), trace=trace)
    return _gather(res.results, vidx, V, Vc), res.exec_time_ns


def kernel(**inputs):
    out, _ = run_traced(trace=False, **inputs)
    return out


# revision 12
# speedup vs baseline: 1.1168x; 1.1168x over previous
"""PointNet MLP (3 x conv1x1+BN+ReLU, final valid-mask) on 8 TRN2 cores.

Sharding: compacted-column parallel. The valid mask keeps ~70% of the
4096*128 = 524288 point-neighbor columns; masked columns are exactly 0 in
the reference output. Host gathers the valid columns, splits them evenly
across 8 cores, device computes only those, host scatters into zeros.

Numerics: plain fp16 weights/activations with f32 PSUM accumulation and
fp16 output (end-to-end rel err ~7e-4 vs the 2e-2 gate).

Device schedule: supersteps of 2048 stream columns (2 block-pair
iterations of 1024), software-pipelined 3 stages deep.  Layers 1/2 use
one joint PSUM tile per superstep (ps1/ps2 [128,1024], single-buffered:
their consumers act1/act2 run FIRST on their engines, so the
write-after-read from the next superstep resolves early); layer 3 uses
per-iteration [128,1024] tiles, double-buffered (its consumer act3 runs
last, so its WAR gets a full superstep of slack).  8 PSUM banks total.
Priority offsets pin the scheduler to:
  PE : mm1 pair(s) | mm2 pair(s-1) | mm3 quad(s-2)  - streams 8
       back-to-back matmuls per superstep at the full column rate
       (a matmul that stalls on a semaphore pays a ~300ns restart)
  ACT: act1j(s) Relu(ps1)->fp16 | act3(even iter of s-2) + bias b3
  DVE: act2j(s-1) max(ps2+b2cat,0)->fp16 | act3(odd iter of s-2)
b1 is folded into mm1 via a ones row (K=7); BN folded on host (f64).
One fp16 [128,1024] DMA out per iteration; host casts and scatters.
"""

import numpy as np

try:
    import concourse.bass as bass
except ImportError:
    import sys

    sys.path.insert(0, "/opt/trn_rl_repo")
    import concourse.bass as bass

import concourse.bacc as bacc

import concourse.mybir as mybir
from concourse import tile
from concourse.bass_utils import run_bass_kernel_spmd

F32 = mybir.dt.float32
F16 = mybir.dt.float16

N_CORES = 8
NPOINT, KNN = 4096, 128
NCOLS = NPOINT * KNN
M = 512
ITER_COLS = 2 * M        # 1024 stream columns per iteration
SS_COLS = 2 * ITER_COLS  # 2048 stream columns per superstep
EPS = 1e-5

_NC_CACHE = {}


def _build_nc(iters):
    assert iters % 2 == 0
    n_ss = iters // 2
    nc = bacc.Bacc("TRN2", target_bir_lowering=False)
    xp_d = nc.declare_dram_parameter("xp", [14, iters * M], F16, isOutput=False)
    w1_d = nc.declare_dram_parameter("lhsT1", [14, 128], F16, isOutput=False)
    w2_d = nc.declare_dram_parameter("lhsT2", [128, 128], F16, isOutput=False)
    w3_d = nc.declare_dram_parameter("lhsT3", [128, 128], F16, isOutput=False)
    bias_d = nc.declare_dram_parameter("biases", [128, 2], F32, isOutput=False)
    out_d = nc.declare_dram_parameter("out", [128, iters * ITER_COLS], F16,
                                      isOutput=True)

    add = mybir.AluOpType.add
    vmax = mybir.AluOpType.max
    relu_fn = mybir.ActivationFunctionType.Relu

    with tile.TileContext(nc) as tc:
        with (
            tc.tile_pool(name="const", bufs=1) as cpool,
            tc.tile_pool(name="xpool", bufs=1) as xpool,
            tc.tile_pool(name="ypool", bufs=3) as ypool,
            tc.tile_pool(name="opool", bufs=4) as opool,
            tc.tile_pool(name="pspool", bufs=1, space="PSUM") as pspool,
            tc.tile_pool(name="ps3pool", bufs=2, space="PSUM") as ps3pool,
        ):
            w1_sb = cpool.tile([39, 128], F16, tag="w1", name="w1_sb")
            w2_sb = cpool.tile([128, 128], F16, tag="w2", name="w2_sb")
            w3_sb = cpool.tile([128, 128], F16, tag="w3", name="w3_sb")
            bias_sb = cpool.tile([128, 2], F32, tag="bias", name="bias_sb")
            b2_ap = bias_sb[:, 0:1]
            b3_ap = bias_sb[:, 1:2]

            xcols = iters * M
            x_sb = xpool.tile([39, xcols], F16, tag="x", name="x_sb")
            # first chunk small so mm1 of superstep 0 starts early
            c0_end = min(xcols, 4 * M)
            nc.sync.dma_start(x_sb[0:7, 0:c0_end], xp_d[0:7, 0:c0_end])
            nc.sync.dma_start(x_sb[32:39, 0:c0_end], xp_d[7:14, 0:c0_end])
            nc.sync.dma_start(w1_sb[0:7, :], w1_d[0:7, :])
            nc.sync.dma_start(w1_sb[32:39, :], w1_d[7:14, :])
            nc.sync.dma_start(w2_sb[:, :], w2_d[:, :])
            nc.sync.dma_start(w3_sb[:, :], w3_d[:, :])
            nc.scalar.dma_start(bias_sb[:, :], bias_d[:, :])
            rest = xcols - c0_end
            nch = 3
            chunk = -(-rest // nch) if rest > 0 else 0
            for c in range(nch):
                lo = c0_end + c * chunk
                hi = min(xcols, lo + chunk)
                if hi > lo:
                    eng = [nc.scalar, nc.scalar, nc.sync][c]
                    eng.dma_start(x_sb[0:7, lo:hi], xp_d[0:7, lo:hi])
                    eng.dma_start(x_sb[32:39, lo:hi], xp_d[7:14, lo:hi])

            T = {}

            def mk(s):
                T[s] = dict(
                    ps1=pspool.tile([128, 2 * M], F32, tag="ps1",
                                    name=f"ps1_{s}"),
                    ps2=pspool.tile([128, 2 * M], F32, tag="ps2",
                                    name=f"ps2_{s}"),
                    ps3a=ps3pool.tile([128, 2 * M], F32, tag="ps3",
                                      name=f"ps3a_{s}"),
                    ps3b=ps3pool.tile([128, 2 * M], F32, tag="ps3",
                                      name=f"ps3b_{s}"),
                    hi1=ypool.tile([128, 2 * M], F16, tag="hi1",
                                   name=f"hi1_{s}"),
                    hi2=ypool.tile([128, 2 * M], F16, tag="hi2",
                                   name=f"hi2_{s}"),
                    oba=opool.tile([128, 2 * M], F16, tag="ob",
                                   name=f"oba_{s}"),
                    obb=opool.tile([128, 2 * M], F16, tag="ob",
                                   name=f"obb_{s}"),
                )

            for s in range(n_ss + 2):
                if s < n_ss:
                    mk(s)
                    d = T[s]
                    c0 = s * 2 * M
                    with tc.high_priority(offset=60):
                        nc.tensor.matmul(d["ps1"][:, 0:M], w1_sb[0:7, :],
                                         x_sb[0:7, c0 : c0 + M])
                        nc.tensor.matmul(d["ps1"][:, M : 2 * M], w1_sb[32:39, :],
                                         x_sb[32:39, c0 + M : c0 + 2 * M])
                if 1 <= s <= n_ss:
                    d = T[s - 1]
                    with tc.high_priority(offset=40):
                        nc.tensor.matmul(d["ps2"][:, 0:M], w2_sb[:, :],
                                         d["hi1"][:, 0:M])
                        nc.tensor.matmul(d["ps2"][:, M : 2 * M], w2_sb[:, :],
                                         d["hi1"][:, M : 2 * M])
                if s >= 2:
                    d = T[s - 2]
                    nc.tensor.matmul(d["ps3a"][:, 0:M], w3_sb[0:64, :],
                                     d["hi2"][0:64, 0:M])
                    nc.tensor.matmul(d["ps3a"][:, M : 2 * M], w3_sb[64:128, :],
                                     d["hi2"][64:128, 0:M])
                    nc.tensor.matmul(d["ps3b"][:, 0:M], w3_sb[0:64, :],
                                     d["hi2"][0:64, M : 2 * M])
                    nc.tensor.matmul(d["ps3b"][:, M : 2 * M], w3_sb[64:128, :],
                                     d["hi2"][64:128, M : 2 * M])

                if s < n_ss:
                    d = T[s]
                    with tc.high_priority(offset=60):
                        nc.scalar.activation(d["hi1"][:, :], d["ps1"][:, :],
                                             relu_fn)
                if 1 <= s <= n_ss:
                    d = T[s - 1]
                    with tc.high_priority(offset=40):
                        nc.scalar.activation(d["hi2"][:, :], d["ps2"][:, :],
                                             relu_fn, bias=b2_ap)
                if s >= 2:
                    d = T[s - 2]
                    nc.vector.tensor_scalar(d["oba"][:, :], d["ps3a"][:, :],
                                            b3_ap, 0.0, add, vmax)
                    nc.vector.tensor_scalar(d["obb"][:, :], d["ps3b"][:, :],
                                            b3_ap, 0.0, add, vmax)
                    o0 = (s - 2) * SS_COLS
                    nc.sync.dma_start(out_d[:, o0 : o0 + 2 * M],
                                      d["oba"][:, :])
                    nc.sync.dma_start(out_d[:, o0 + 2 * M : o0 + 4 * M],
                                      d["obb"][:, :])
                    del T[s - 2]

    nc.compile()
    return nc


def _get_nc(iters):
    if iters not in _NC_CACHE:
        _NC_CACHE[iters] = _build_nc(iters)
    return _NC_CACHE[iters]


def _fold_bn(W, b, gamma, beta, mean, var):
    inv = gamma.astype(np.float64) / np.sqrt(var.astype(np.float64) + EPS)
    Wp = (W.astype(np.float64) * inv[:, None]).astype(np.float32)
    bp = ((b.astype(np.float64) - mean.astype(np.float64)) * inv
          + beta.astype(np.float64)).astype(np.float32)
    return Wp, bp


def _prepare(inputs):
    gp = np.asarray(inputs["grouped_pc"], dtype=np.float32)
    valid = np.asarray(inputs["valid"], dtype=np.float32)

    Wp1, bp1 = _fold_bn(*(np.asarray(inputs[k], dtype=np.float32)
                          for k in ("W1", "b1", "gamma1", "beta1", "mean1", "var1")))
    Wp2, bp2 = _fold_bn(*(np.asarray(inputs[k], dtype=np.float32)
                          for k in ("W2", "b2", "gamma2", "beta2", "mean2", "var2")))
    Wp3, bp3 = _fold_bn(*(np.asarray(inputs[k], dtype=np.float32)
                          for k in ("W3", "b3", "gamma3", "beta3", "mean3", "var3")))

    lhsT1 = np.zeros((14, 128), np.float16)
    lhsT1[0:3, 0:64] = Wp1.T
    lhsT1[3:6, 64:128] = Wp1.T
    lhsT1[6, 0:64] = bp1
    lhsT1[6, 64:128] = bp1
    lhsT1[7:14] = lhsT1[0:7]

    lhsT2 = np.zeros((128, 128), np.float16)
    lhsT2[0:64, 0:64] = Wp2.T
    lhsT2[64:128, 64:128] = Wp2.T

    lhsT3 = np.zeros((128, 128), np.float16)
    lhsT3[0:64, :] = Wp3.T
    lhsT3[64:128, :] = Wp3.T

    biases = np.zeros((128, 2), np.float32)
    biases[:, 0] = np.concatenate([bp2, bp2])
    biases[:, 1] = bp3

    x = gp[0].reshape(3, NCOLS)
    vidx = np.flatnonzero(valid.reshape(NCOLS) > 0.5)
    V = len(vidx)
    Vc = -(-V // N_CORES)
    iters = max(2, 2 * (-(-Vc // SS_COLS)))
    cap = iters * ITER_COLS

    xv = x[:, vidx].astype(np.float16)

    in_maps = []
    for c in range(N_CORES):
        lo_i = c * Vc
        hi_i = min((c + 1) * Vc, V)
        n = max(0, hi_i - lo_i)
        xa = np.zeros((3, cap), np.float16)
        if n:
            xa[:, :n] = xv[:, lo_i:hi_i]
        xr = xa.reshape(3, iters, 2, M)
        xp = np.ones((14, iters, M), np.float16)
        xp[0:3] = xr[:, :, 0, :]
        xp[3:6] = xr[:, :, 1, :]
        xp[7:14] = xp[0:7]
        in_maps.append(
            {
                "xp": np.ascontiguousarray(xp.reshape(14, iters * M)),
                "lhsT1": lhsT1,
                "lhsT2": lhsT2,
                "lhsT3": lhsT3,
                "biases": biases,
            }
        )
    return in_maps, vidx, V, Vc, iters


def _gather(results, vidx, V, Vc):
    stream = np.empty((128, V), np.float32)
    for c in range(N_CORES):
        lo_i = c * Vc
        hi_i = min((c + 1) * Vc, V)
        if hi_i <= lo_i:
            break
        stream[:, lo_i:hi_i] = results[c]["out"][:, : hi_i - lo_i]
    full = np.zeros((128, NCOLS), np.float32)
    full[:, vidx] = stream
    return full.reshape(128, NPOINT, KNN)[None]


def run_traced(trace=False, **inputs):
    in_maps, vidx, V, Vc, iters = _prepare(inputs)
    nc = _get_nc(iters)
    res = run_bass_kernel_spmd(nc, in_maps, list(range(N_CORES)), trace=trace)
    return _gather(res.results, vidx, V, Vc), res.exec_time_ns


def kernel(**inputs):
    out, _ = run_traced(trace=False, **inputs)
    return out
